# revision 7
# baseline (speedup 1.0000x reference)
"""Fake-attention kernel for trn2: 8 NeuronCores, one batch element per core.

Per core (batch b): out = softmax(k @ q^T) @ v, with k/q/v = x @ W.T + b.

Dataflow (transposed so the PV contraction lands on partitions):
  xT [f,n]     <- transposed on host; loaded via gpsimd casting DMAs so
                  the tiles carry fp32r (full-rate PE) directly
  kT,qT [d,n]  = W @ xT (fp32r) + bias at the PSUM->SBUF copy-out (DVE)
  v [m,d]      = xT-chunks as lhsT, rhs = Wv^T (fp32r); the copy-out adds
                 bv broadcast along d and writes BF16 (bv rides the PV sum
                 exactly because softmax weights sum to 1)
  per n-section of 1024, m-chunks of 128 (PV lags scores by 2 chunks):
    scoresT chunk [m=128, n=1024] = qT-slice as lhsT, kT as rhs (fp32r)
    pT = exp(scoresT) in BF16 (ACT, PSUM->SBUF convert-on-write)
    outT [d,n] += v-chunk(bf16) as lhsT, pT(bf16) as rhs (PSUM acc over m)
    denominator: secs 1-3 column-split - DVE adds cols [0:640] in BF16
    (2x_1p rate, two ping-pong accumulators for precision), GPSIMD adds
    cols [640:1024] into an fp32 accumulator (GPSIMD cost is dtype-blind);
    sec 0 alternates whole chunks (GPS even / DVE odd, BF16 accumulators)
    so DVE's odd-chunk slack absorbs the setup projection copy-outs
  finalize (spread over the NEXT section's chunk stream):
    o_copy = PSUM outT -> SBUF BF16 halves
    denom[n] per 128-block via ones-matmuls (bf16/f32r ones to match the
    accumulator dtypes; split accumulators merge in PSUM) -> reciprocal
    out natural = PE-transpose blocks (bf16, full rate even at 128 cols);
    out = tpo * recip_n via one DVE tensor_scalar_mul per block (bias bv
    already folded into v)
  last section: tight serial tail - the ones/recip denominator chain gets
  its own sc-pool tile; ACT scales half the blocks (Copy+scale-AP) while
  DVE ts_muls the rest, and the two output DMAs split over SP/ACT queues.

PSUM (8 banks): sc pool 2x[128,1024]f32 (4) + pv 1x (2) + aux 1x (2).
Section PV accumulators ping-pong pv/aux so boundaries have no PSUM
coupling; sec-0 setup projections and per-section finalize tiles reuse
whichever accumulator pool is idle at that point in the schedule.
"""
import numpy as np

B = 8
N = 4096
D = 128
NC = 32          # chunks of 128 along n/m
NSEC = 4         # sections of 1024 along n
SEC = 1024
DCOL = 640       # denominator cols on DVE (d_even); rest on GPSIMD (d_odd)
XSPL = 768       # exp split: ACT gets XSPL cols, DVE-Schraudolph the rest
                 # (window alternates sides per chunk to spread the error)
SCH_A = 184.6646884   # 2^7 * log2(e): Schraudolph exp in bf16 bit space
SCH_B = 16248.0       # (127 - c) * 2^7, calibrated on the real score range

_cache = {}


def _build(dcol=DCOL, warmup_mms=16, ptp_bufs=5):
    import concourse.bass as bass  # noqa
    import concourse.mybir as mybir
    import concourse.tile as tile
    from concourse import bacc

    F32 = mybir.dt.float32
    F32R = mybir.dt.float32r
    BF16 = mybir.dt.bfloat16
    I16 = mybir.dt.int16
    Exp = mybir.ActivationFunctionType.Exp
    Ident = mybir.ActivationFunctionType.Identity
    ADD = mybir.AluOpType.add
    MULT = mybir.AluOpType.mult
    ecol = SEC - dcol
    assert dcol % 128 == 0
    nde = dcol // 128

    nc = bacc.Bacc()
    xt = nc.declare_dram_parameter("xt", [D, N], F32, isOutput=False)
    wp = nc.declare_dram_parameter("wp", [128, 643], F32, isOutput=False)
    y = nc.declare_dram_parameter("y", [N, D], F32, isOutput=True)

    xt_dram = xt.rearrange("p (c l) -> p c l", l=128)
    y_dram = y.rearrange("(c p) d -> p c d", p=128)

    with tile.TileContext(nc) as tc:
        with (
            tc.tile_pool(name="big", bufs=1) as big,
            tc.tile_pool(name="ptp", bufs=ptp_bufs) as ptp,
            tc.tile_pool(name="wrk", bufs=2) as wrk,
            tc.tile_pool(name="sc", bufs=2, space="PSUM") as ps_sc,
            tc.tile_pool(name="pv", bufs=1, space="PSUM") as ps_pv,
            tc.tile_pool(name="aux", bufs=1, space="PSUM") as ps_aux,
        ):
            # --- input DMAs, criticality-ordered -------------------------
            xg0a = big.tile([128, 4, 128], F32R, tag="xT0a")
            xg0b = big.tile([128, 4, 128], F32R, tag="xT0b")
            wf32 = big.tile([128, 512], F32, tag="wf32")
            wbig = big.tile([128, 512], F32R, tag="wbig")
            wrest = big.tile([128, 131], F32, tag="wrest")
            wsm = big.tile([128, 258], BF16, tag="wsm")
            # x tiles come in via gpsimd casting DMAs (swdge queue); the
            # weights via the SP queue as fp32 plus a DVE rounding copy
            nc.gpsimd.dma_start(xg0a[:], xt_dram[:, 0:4, :])
            nc.gpsimd.dma_start(xg0b[:], xt_dram[:, 4:8, :])
            # startup-critical wk/wq land in their own small DMA first
            nc.sync.dma_start(wf32[:, 0:256], wp[:, 0:256])
            nc.sync.dma_start(wrest[:], wp[:, 512:643])
            nc.sync.dma_start(wf32[:, 256:512], wp[:, 256:512])
            # rounding copies: critical wq/wk now; wv/ident after the
            # prologue's k-chain (they are only needed for v0 / finalize)
            nc.vector.tensor_copy(wbig[:, 128:256], wf32[:, 128:256])
            nc.vector.tensor_copy(wbig[:, 0:128], wf32[:, 0:128])
            wkT = wbig[:, 0:128]
            wqT = wbig[:, 128:256]
            wvT = wbig[:, 256:384]
            ident_bf = wsm[:, 0:128]
            ones_bf = wsm[:, 128:129]
            bv_bc = wrest[:, 1:129]
            bk = wrest[:, 129:130]
            bq = wrest[:, 130:131]
            ones_col = wrest[:, 0:1]

            xT_g = [None] * 4
            xT_g[0] = (xg0a, xg0b)

            def emit_dma_x(g):
                xg = big.tile([128, 8, 128], F32R, tag=f"xT{g}")
                nc.gpsimd.dma_start(xg[:], xt_dram[:, g * 8:(g + 1) * 8, :])
                xT_g[g] = xg

            emit_dma_x(1)

            wu = big.tile([128, 128], BF16, tag="warm")
            nc.vector.memset(wu[:], 1.0)
            wu_ps = ps_sc.tile([128, 1024], F32, tag="sc")
            for _ in range(warmup_mms):
                nc.tensor.matmul(wu_ps[:, 0:128], wu[:], wu[:],
                                 start=True, stop=True,
                                 skip_group_check=True)

            def ptile(pool):
                t = pool.tile([128, 1024], F32,
                              tag="pvt" if pool is ps_pv else "auxt")
                return t

            def xslab(g, half):
                xg = xT_g[g]
                if isinstance(xg, tuple):
                    return xg[half].rearrange("p c f -> p (c f)")
                return xg.rearrange("p c f -> p (c f)")[
                    :, half * 512:(half + 1) * 512]

            def xchunk(g, j):
                xg = xT_g[g]
                if isinstance(xg, tuple):
                    return xg[j // 4][:, j % 4, :]
                return xg[:, j, :]

            kT = [None] * 4
            qT = [None] * 4
            v_g = [None] * 4

            # --- projection helpers (setup); psum from an explicit pool --
            def emit_k(g, pool, split=False):
                if split:
                    kga = big.tile([128, 512], F32R, tag="kT0a")
                    kgb = big.tile([128, 512], F32R, tag="kT0b")
                    pst = ptile(pool)
                    nc.tensor.matmul(pst[:, 0:512], wkT, xslab(g, 0),
                                     start=True, stop=True)
                    nc.vector.tensor_scalar_add(kga[:], pst[:, 0:512], bk)
                    nc.tensor.matmul(pst[:, 512:1024], wkT, xslab(g, 1),
                                     start=True, stop=True)
                    nc.vector.tensor_scalar_add(kgb[:], pst[:, 512:1024], bk)
                    kT[g] = (kga, kgb)
                    return
                tg = big.tile([128, 1024], F32R, tag=f"kT{g}")
                pst = ptile(pool)
                nc.tensor.matmul(pst[:, 0:512], wkT, xslab(g, 0),
                                 start=True, stop=True)
                nc.tensor.matmul(pst[:, 512:1024], wkT, xslab(g, 1),
                                 start=True, stop=True)
                nc.scalar.activation(tg[:], pst[:], Ident, bias=bk)
                kT[g] = tg

            def v_bias_copy(vg, psv, lo, hi):
                # psv [m,j,d] + bv (broadcast along d) -> vg BF16
                n = hi - lo
                bvx = bv_bc[:, None, :].to_broadcast((128, n, 128))
                nc.vector.tensor_tensor(
                    vg[:, lo:hi, :], psv[:, lo:hi, :], bvx, ADD)

            def emit_v(g, pool):
                vg = big.tile([128, 8, 128], BF16, tag=f"v{g}")
                psv = ptile(pool)
                psv3 = psv.rearrange("p (c f) -> p c f", f=128)
                for j in range(8):
                    nc.tensor.matmul(
                        psv[:, j * 128:(j + 1) * 128], xchunk(g, j), wvT,
                        start=True, stop=True,
                    )
                v_bias_copy(vg, psv3, 0, 8)
                v_g[g] = vg

            def q_slice(mc):
                return qT[mc // 8][:, (mc % 8) * 128:(mc % 8 + 1) * 128]

            def v_chunk(mc):
                return v_g[mc // 8][:, mc % 8, :]

            # --- prologue ------------------------------------------------
            # q0[0:512] rushed into the warmup sc tile's spare columns so
            # nothing serializes behind the k projection's PSUM tile.
            qt0 = big.tile([128, 1024], F32R, tag="qT0")
            qT[0] = qt0
            nc.tensor.matmul(wu_ps[:, 512:640], wqT,
                             xslab(0, 0)[:, 0:128], start=True, stop=True)
            nc.vector.tensor_scalar_add(qt0[:, 0:128], wu_ps[:, 512:640], bq)
            nc.tensor.matmul(wu_ps[:, 640:1024], wqT,
                             xslab(0, 0)[:, 128:512], start=True, stop=True)
            # k0 (ps_pv pool; released before sec-0 PV begins)
            kga = big.tile([128, 512], F32R, tag="kT0a")
            kgb = big.tile([128, 512], F32R, tag="kT0b")
            k0ps = ptile(ps_pv)
            nc.tensor.matmul(k0ps[:, 0:512], wkT, xslab(0, 0),
                             start=True, stop=True)
            nc.vector.tensor_scalar_add(kga[:], k0ps[:, 0:512], bk)
            kT[0] = (kga, kgb)
            # scores chunk 0 first half + exp
            q_sl0 = q_slice(0)
            s0a = ps_sc.tile([128, 1024], F32, tag="sc")
            nc.tensor.matmul(s0a[:, 0:512], q_sl0, kga[:], start=True, stop=True)
            pT0 = ptp.tile([128, 1024], I16, tag="pt")
            pT0b = pT0.bitcast(BF16)
            nc.scalar.activation(pT0b[:, 0:512], s0a[:, 0:512], Exp)
            # k0 second half + scores 0 second half (before v0: x0b path)
            nc.tensor.matmul(k0ps[:, 512:1024], wkT, xslab(0, 1),
                             start=True, stop=True)
            nc.vector.tensor_scalar_add(kgb[:], k0ps[:, 512:1024], bk)
            s0b = ps_sc.tile([128, 1024], F32, tag="sc")
            nc.tensor.matmul(s0b[:, 0:512], q_sl0, kgb[:], start=True, stop=True)
            nc.scalar.activation(pT0b[:, 512:1024], s0b[:, 0:512], Exp)
            # deferred weight casts + v0 (needed by PV(0) at iteration 2);
            # q0 tail bias-adds follow (needed from scores chunk 1 onward)
            nc.vector.tensor_copy(wbig[:, 256:384], wf32[:, 256:384])
            nc.vector.tensor_copy(ident_bf, wf32[:, 384:512])
            nc.vector.memset(ones_bf, 1.0)
            emit_v(0, ps_aux)
            nc.vector.tensor_scalar_add(qt0[:, 128:512], wu_ps[:, 640:1024], bq)
            # q0 tail (aux, after v0)
            q0ps = ptile(ps_aux)
            nc.tensor.matmul(q0ps[:, 512:1024], wqT, xslab(0, 1),
                             start=True, stop=True)
            nc.vector.tensor_scalar_add(qt0[:, 512:1024], q0ps[:, 512:1024], bq)

            # per-section state carried into the next section's stream
            fin_jobs = [None]   # list of (step_fn) queue for spread finalize

            # sec-0 setup schedule: (mc -> callable)
            setup_sched = {}

            def add_setup(mc, fn):
                setup_sched.setdefault(mc, []).append(fn)

            # sec-0 setup, two-phase: matmuls at an even chunk (PE slack),
            # the DVE copy/bias-add at the following odd chunks — odd
            # iterations carry a GPSIMD denominator, so DVE is free there.
            setup_state = {}

            def emit_q_mm(g):
                tg = big.tile([128, 1024], F32R, tag=f"qT{g}")
                pst = ptile(ps_aux)
                nc.tensor.matmul(pst[:, 0:512], wqT, xslab(g, 0),
                                 start=True, stop=True)
                nc.tensor.matmul(pst[:, 512:1024], wqT, xslab(g, 1),
                                 start=True, stop=True)
                setup_state[f"q{g}"] = (tg, pst)
                qT[g] = tg

            def emit_q_add(g):
                tg, pst = setup_state.pop(f"q{g}")
                nc.scalar.activation(tg[:], pst[:], Ident, bias=bq)

            def emit_k_mm(g):
                tg = big.tile([128, 1024], F32R, tag=f"kT{g}")
                pst = ptile(ps_aux)
                nc.tensor.matmul(pst[:, 0:512], wkT, xslab(g, 0),
                                 start=True, stop=True)
                nc.tensor.matmul(pst[:, 512:1024], wkT, xslab(g, 1),
                                 start=True, stop=True)
                setup_state[f"k{g}"] = (tg, pst)
                kT[g] = tg

            def emit_k_add(g):
                tg, pst = setup_state.pop(f"k{g}")
                nc.scalar.activation(tg[:], pst[:], Ident, bias=bk)

            def emit_v_mm(g, quarter):
                if quarter == 0:
                    vg = big.tile([128, 8, 128], BF16, tag=f"v{g}")
                    psv = ptile(ps_aux)
                    setup_state[f"v{g}"] = (vg, psv)
                    v_g[g] = vg
                else:
                    vg, psv = setup_state[f"v{g}"]
                for j in range(quarter * 2, quarter * 2 + 2):
                    nc.tensor.matmul(
                        psv[:, j * 128:(j + 1) * 128], xchunk(g, j), wvT,
                        start=True, stop=True,
                    )

            def emit_v_copy(g):
                vg, psv = setup_state.pop(f"v{g}")
                psv3 = psv.rearrange("p (c f) -> p c f", f=128)
                v_bias_copy(vg, psv3, 0, 8)

            add_setup(1, lambda: emit_q_mm(1))
            add_setup(1, lambda: emit_dma_x(2))
            add_setup(3, lambda: emit_q_add(1))
            add_setup(3, lambda: emit_dma_x(3))
            for s, qq in ((4, 0), (5, 1), (6, 2), (7, 3)):
                add_setup(s, lambda q=qq: emit_v_mm(1, q))
            add_setup(9, lambda: emit_v_copy(1))
            add_setup(9, lambda: emit_q_mm(2))
            add_setup(11, lambda: emit_q_add(2))
            for s, qq in ((12, 0), (13, 1), (14, 2), (15, 3)):
                add_setup(s, lambda q=qq: emit_v_mm(2, q))
            add_setup(17, lambda: emit_v_copy(2))
            add_setup(17, lambda: emit_q_mm(3))
            add_setup(19, lambda: emit_q_add(3))
            for s, qq in ((20, 0), (21, 1), (22, 2), (23, 3)):
                add_setup(s, lambda q=qq: emit_v_mm(3, q))
            add_setup(25, lambda: emit_v_copy(3))
            add_setup(25, lambda: emit_k_mm(1))
            add_setup(27, lambda: emit_k_add(1))

            # deferred k projections: k2 in sec 1, k3 in sec 2 (pool chosen
            # to be the one NOT accumulating that section's PV)
            def sec_pool(sec):
                return ps_pv if sec % 2 == 0 else ps_aux

            pT_prev = pT0
            last_pv_chain = [None]  # pending last-chunk PV+denoms of section

            for sec in range(NSEC):
                # column split everywhere: DVE cols [0:dcol] (bf16, two
                # ping-pong accumulators), GPS cols [dcol:] (fp32 acc).
                d_acc_a = wrk.tile([128, dcol], BF16, tag="dea")
                d_acc_b = wrk.tile([128, dcol], BF16, tag="deb")
                d_acc = [d_acc_a, d_acc_b]
                d_odd = wrk.tile([128, ecol], F32, tag="do")
                ps_pv_t = ptile(sec_pool(sec))

                def emit_scores(mc, sec=sec):
                    ps_s = ps_sc.tile([128, 1024], F32, tag="sc")
                    q_sl = q_slice(mc)
                    kg = kT[sec]
                    if isinstance(kg, tuple):
                        ka, kb = kg[0][:], kg[1][:]
                    else:
                        ka, kb = kg[:, 0:512], kg[:, 512:1024]
                    nc.tensor.matmul(ps_s[:, 0:512], q_sl, ka,
                                     start=True, stop=True)
                    nc.tensor.matmul(ps_s[:, 512:1024], q_sl, kb,
                                     start=True, stop=True)
                    return ps_s

                ecols = SEC - XSPL

                def emit_exp(ps_s, mc, split=False):
                    # every chunk: ACT exps XSPL cols while DVE computes the
                    # remaining cols as a Schraudolph bf16-bit pattern (one
                    # fused mult+add with int16 convert-on-write). The DVE
                    # window alternates sides per chunk so each output row
                    # only has half its weights approximated.
                    pT = ptp.tile([128, 1024], I16, tag="pt")
                    pTb = pT.bitcast(BF16)
                    if mc % 2 == 0:
                        nc.vector.tensor_scalar(
                            pT[:, XSPL:SEC], ps_s[:, XSPL:SEC],
                            SCH_A, SCH_B, MULT, ADD)
                        if split:
                            nc.scalar.activation(pTb[:, 0:512], ps_s[:, 0:512], Exp)
                            nc.scalar.activation(pTb[:, 512:XSPL],
                                                 ps_s[:, 512:XSPL], Exp)
                        else:
                            nc.scalar.activation(pTb[:, 0:XSPL],
                                                 ps_s[:, 0:XSPL], Exp)
                    else:
                        nc.vector.tensor_scalar(
                            pT[:, 0:ecols], ps_s[:, 0:ecols],
                            SCH_A, SCH_B, MULT, ADD)
                        if split:
                            nc.scalar.activation(pTb[:, ecols:512],
                                                 ps_s[:, ecols:512], Exp)
                            nc.scalar.activation(pTb[:, 512:1024],
                                                 ps_s[:, 512:1024], Exp)
                        else:
                            nc.scalar.activation(pTb[:, ecols:SEC],
                                                 ps_s[:, ecols:SEC], Exp)
                    return pT

                def emit_pv(mc, pT, ps_pv_t=ps_pv_t):
                    pTb = pT.bitcast(BF16)
                    nc.tensor.matmul(
                        ps_pv_t[:, 0:512], v_chunk(mc), pTb[:, 0:512],
                        start=(mc == 0), stop=(mc == NC - 1),
                        skip_group_check=True,
                    )
                    nc.tensor.matmul(
                        ps_pv_t[:, 512:1024], v_chunk(mc), pTb[:, 512:1024],
                        start=(mc == 0), stop=(mc == NC - 1),
                        skip_group_check=True,
                    )

                def emit_denom(mc, pT, d_odd=d_odd, d_acc=d_acc, sec=sec):
                    pTb = pT.bitcast(BF16)
                    da = d_acc[mc % 2]
                    if mc < 2:
                        nc.vector.tensor_copy(da[:], pTb[:, 0:dcol])
                    else:
                        nc.vector.tensor_tensor(
                            da[:], da[:], pTb[:, 0:dcol], ADD)
                    if mc == 0:
                        nc.gpsimd.tensor_copy(d_odd[:], pTb[:, dcol:SEC])
                    else:
                        nc.gpsimd.tensor_tensor(
                            d_odd[:], d_odd[:], pTb[:, dcol:SEC], ADD)

                # finalize of THIS section (runs spread over next section)
                def make_fin(sec=sec, ps_pv_t=ps_pv_t,
                             d_odd=d_odd, d_acc=d_acc):
                    st = {}
                    pool = sec_pool(sec)
                    last = (sec == NSEC - 1)

                    def get_oc():
                        if "oc" not in st:
                            oc = wrk.tile([128, 1024], BF16, tag="oc")
                            st["oc"] = oc
                        return st["oc"]

                    def f_ocopy_a():
                        oc = get_oc()
                        if last:
                            nc.scalar.copy(oc[:, 0:512], ps_pv_t[:, 0:512])
                        else:
                            nc.vector.tensor_copy(oc[:, 0:512], ps_pv_t[:, 0:512])

                    def f_ocopy_b():
                        oc = get_oc()
                        if last:
                            # ACT again: DVE is busy with the final denom adds
                            nc.scalar.copy(oc[:, 512:1024], ps_pv_t[:, 512:1024])
                        else:
                            nc.vector.tensor_copy(oc[:, 512:1024], ps_pv_t[:, 512:1024])

                    def emit_denom_mms(ft):
                        for nb in range(8):
                            sl = slice(nb * 128, (nb + 1) * 128)
                            if nb < nde:
                                nc.tensor.matmul(
                                    ft[:, nb:nb + 1], d_acc[0][:, sl], ones_bf,
                                    start=True, stop=False,
                                    skip_group_check=True,
                                )
                                nc.tensor.matmul(
                                    ft[:, nb:nb + 1], d_acc[1][:, sl], ones_bf,
                                    start=False, stop=True,
                                    skip_group_check=True,
                                )
                            else:
                                sl2 = slice((nb - nde) * 128, (nb - nde + 1) * 128)
                                nc.tensor.matmul(
                                    ft[:, nb:nb + 1], d_odd[:, sl2], ones_col,
                                    start=True, stop=True,
                                    skip_group_check=True,
                                )

                    def f_denom():
                        ft = ptile(pool)
                        st["ft"] = ft
                        emit_denom_mms(ft)
                        rc = wrk.tile([128, 8], F32, tag="rc")
                        nc.vector.reciprocal(rc[:], ft[:, 0:8])
                        st["rc"] = rc
                        og = big.tile([128, 8, 128], F32, tag=f"out{sec % 2}")
                        st["og"] = og

                    def tpo_view(ft, nb):
                        # bf16 [128,128] transpose target inside the f32 tile
                        lo = 64 + nb * 64
                        return ft[:, lo:lo + 64].bitcast(BF16)

                    def mk_block(nb):
                        def f_block():
                            ft = st["ft"]
                            oc = st["oc"]
                            rc = st["rc"]
                            og = st["og"]
                            sl = slice(nb * 128, (nb + 1) * 128)
                            tp = tpo_view(ft, nb)
                            nc.tensor.transpose(tp, oc[:, sl], ident_bf)
                            nc.vector.tensor_scalar_mul(
                                og[:, nb, :], tp, rc[:, nb:nb + 1])
                        return f_block

                    def mk_dma(quarter):
                        def f_dma():
                            og = st["og"]
                            lo, hi = quarter * 2, quarter * 2 + 2
                            nc.sync.dma_start(
                                y_dram[:, sec * 8 + lo:sec * 8 + hi, :],
                                og[:, lo:hi, :],
                            )
                        return f_dma

                    if not last:
                        steps = [f_ocopy_a, f_ocopy_b, f_denom]
                        for nb in range(8):
                            steps.append(mk_block(nb))
                            if nb % 2 == 1:
                                steps.append(mk_dma(nb // 2))
                        return steps

                    # last section: tight serial tail. Transposes land in the
                    # now-idle sc pool (bf16 views); the denom chain gets its
                    # own sc tile so it runs in parallel with the copies.
                    def mk_tr(half):
                        def f_tr():
                            tpo = ps_sc.tile([128, 1024], F32, tag="sc")
                            st[f"tpo{half}"] = tpo
                            oc = st["oc"]
                            for j in range(4):
                                nb = half * 4 + j
                                tp = tpo[:, j * 64:(j + 1) * 64].bitcast(BF16)
                                nc.tensor.transpose(
                                    tp, oc[:, nb * 128:(nb + 1) * 128],
                                    ident_bf)
                        return f_tr

                    def f_denom_last():
                        ftd = ps_sc.tile([128, 1024], F32, tag="sc")
                        emit_denom_mms(ftd)
                        rc = wrk.tile([128, 8], F32, tag="rc")
                        nc.vector.reciprocal(rc[:], ftd[:, 0:8])
                        st["rc"] = rc
                        og = big.tile([128, 8, 128], F32, tag=f"out{sec % 2}")
                        st["og"] = og

                    def mk_scale_dve(nb):
                        def f_s():
                            tpo = st[f"tpo{nb // 4}"]
                            j = nb % 4
                            tp = tpo[:, j * 64:(j + 1) * 64].bitcast(BF16)
                            nc.vector.tensor_scalar_mul(
                                st["og"][:, nb, :], tp, st["rc"][:, nb:nb + 1])
                        return f_s

                    def mk_scale_act(nb):
                        def f_scale():
                            tpo = st[f"tpo{nb // 4}"]
                            j = nb % 4
                            tp = tpo[:, j * 64:(j + 1) * 64].bitcast(BF16)
                            nc.scalar.mul(
                                st["og"][:, nb, :], tp, st["rc"][:, nb:nb + 1])
                        return f_scale

                    def mk_dma4(half, eng):
                        def f_dma():
                            og = st["og"]
                            lo, hi = half * 4, half * 4 + 4
                            eng().dma_start(
                                y_dram[:, sec * 8 + lo:sec * 8 + hi, :],
                                og[:, lo:hi, :],
                            )
                        return f_dma

                    def mk_dma2(quarter, eng):
                        def f_dma():
                            og = st["og"]
                            lo, hi = quarter * 2, quarter * 2 + 2
                            eng().dma_start(
                                y_dram[:, sec * 8 + lo:sec * 8 + hi, :],
                                og[:, lo:hi, :],
                            )
                        return f_dma

                    # pipelined tail: first half's transpose/scale/DMA chase
                    # ocopy_a while ACT copies the second half
                    steps = [f_denom_last, f_ocopy_a, mk_tr(0), f_ocopy_b,
                             mk_scale_dve(0), mk_scale_dve(1),
                             mk_scale_act(2), mk_scale_act(3),
                             mk_dma2(0, lambda: nc.sync),
                             mk_tr(1),
                             mk_dma2(1, lambda: nc.scalar),
                             mk_scale_dve(4), mk_scale_dve(5),
                             mk_scale_act(6), mk_scale_act(7),
                             mk_dma2(2, lambda: nc.sync),
                             mk_dma2(3, lambda: nc.scalar)]
                    return steps

                # positions in the NEXT section where fin steps run
                fin_positions = [2, 3, 4, 6, 7, 8, 9, 10, 11, 12, 13, 14, 15,
                                 16, 17]

                # PV lags scores by 2 chunks so the PE always runs the next
                # scores matmul first — its completion sem then settles well
                # before ACT needs it, keeping exp cadence at engine rate.
                pT_hist = {}
                if sec == 0:
                    pT_hist[0] = pT_prev
                for mc in range(NC):
                    if not (mc == 0 and sec == 0):
                        ps_s = emit_scores(mc)
                    # carried-over PV/denom of the previous section
                    if mc <= 1 and last_pv_chain[0]:
                        last_pv_chain[0][mc]()
                    if mc >= 2:
                        emit_pv(mc - 2, pT_hist.pop(mc - 2))
                    if mc >= 1:
                        emit_denom(mc - 1, pT_hist[mc - 1])
                    if not (mc == 0 and sec == 0):
                        pT_hist[mc] = emit_exp(ps_s, mc)
                    if mc >= 1:
                        # spread: sec-0 setup / fin steps / deferred k
                        if sec == 0 and mc in setup_sched:
                            for fn in setup_sched[mc]:
                                fn()
                        if sec > 0 and fin_jobs[0]:
                            if mc in fin_positions:
                                idx = fin_positions.index(mc)
                                if idx < len(fin_jobs[0]):
                                    fin_jobs[0][idx]()
                        if sec == 1 and mc == 20:
                            emit_k(2, ps_pv)
                        elif sec == 2 and mc == 20:
                            emit_k(3, ps_aux)

                # pend the last two PVs + last denom into the next section
                def mk_last(p30=pT_hist[NC - 2], p31=pT_hist[NC - 1],
                            pv=emit_pv, dn=emit_denom):
                    def run0():
                        pv(NC - 2, p30)

                    def run1():
                        pv(NC - 1, p31)
                        dn(NC - 1, p31)
                    return [run0, run1]

                last_pv_chain[0] = mk_last()
                fin_jobs[0] = make_fin()

            # drain: last section's PV/denoms + its finalize immediately
            if last_pv_chain[0] is not None:
                for fn in last_pv_chain[0]:
                    fn()
                last_pv_chain[0] = None
            for step in fin_jobs[0]:
                step()

    nc.finalize()
    return nc


def _get_nc():
    if "nc" not in _cache:
        _cache["nc"] = _build()
    return _cache["nc"]


def make_wp(Wk, Wq, Wv, bk, bq, bv):
    # layout: startup-critical wk/wq first so their DMA can land alone
    wp = np.zeros((128, 643), np.float32)
    wp[:, 0:128] = Wk.T
    wp[:, 128:256] = Wq.T
    wp[:, 256:384] = Wv.T
    wp[:, 384:512] = np.eye(128, dtype=np.float32)
    wp[:, 512] = 1.0
    wp[:, 513:641] = np.broadcast_to(bv[None, :], (128, 128))
    wp[:, 641] = bk
    wp[:, 642] = bq
    return wp


def kernel(x, Wk, bk, Wq, bq, Wv, bv, **_ignored):
    from concourse.bass_utils import run_bass_kernel_spmd

    x = np.asarray(x, dtype=np.float32)
    wp = make_wp(
        np.asarray(Wk, np.float32), np.asarray(Wq, np.float32),
        np.asarray(Wv, np.float32), np.asarray(bk, np.float32),
        np.asarray(bq, np.float32), np.asarray(bv, np.float32),
    )

    nc = _get_nc()
    in_maps = [
        {"xt": np.ascontiguousarray(x[b].T), "wp": wp} for b in range(B)
    ]
    res = run_bass_kernel_spmd(nc, in_maps, core_ids=list(range(B)))
    out = np.stack([res.results[b]["y"] for b in range(B)], axis=0)
    return out


# revision 10
# speedup vs baseline: 1.1711x; 1.1711x over previous
"""Fake-attention kernel for trn2: 8 NeuronCores, one batch element per core.

Per core (batch b): out = softmax(k @ q^T) @ v, with k/q/v = x @ W.T + b.

Dataflow (transposed so the PV contraction lands on partitions):
  xT [f,n]     <- transposed on host; loaded via gpsimd casting DMAs so
                  the tiles carry fp32r (full-rate PE) directly
  kT,qT [d,n]  = W @ xT (fp32r) + bias at the PSUM->SBUF copy-out (DVE)
  v [m,d]      = xT-chunks as lhsT, rhs = Wv^T (fp32r); the copy-out adds
                 bv broadcast along d and writes BF16 (bv rides the PV sum
                 exactly because softmax weights sum to 1)
  per n-section of 1024, m-chunks of 128 (PV lags scores by 2 chunks):
    scoresT chunk [m=128, n=1024] = qT-slice as lhsT, kT as rhs (fp32r)
    pT = exp(scoresT) in BF16 (ACT, PSUM->SBUF convert-on-write)
    outT [d,n] += v-chunk(bf16) as lhsT, pT(bf16) as rhs (PSUM acc over m)
    denominator: secs 1-3 column-split - DVE adds cols [0:640] in BF16
    (2x_1p rate, two ping-pong accumulators for precision), GPSIMD adds
    cols [640:1024] into an fp32 accumulator (GPSIMD cost is dtype-blind);
    sec 0 alternates whole chunks (GPS even / DVE odd, BF16 accumulators)
    so DVE's odd-chunk slack absorbs the setup projection copy-outs
  finalize (spread over the NEXT section's chunk stream):
    o_copy = PSUM outT -> SBUF BF16 halves
    denom[n] per 128-block via ones-matmuls (bf16/f32r ones to match the
    accumulator dtypes; split accumulators merge in PSUM) -> reciprocal
    out natural = PE-transpose blocks (bf16, full rate even at 128 cols);
    out = tpo * recip_n via one DVE tensor_scalar_mul per block (bias bv
    already folded into v)
  last section: tight serial tail - the ones/recip denominator chain gets
  its own sc-pool tile; ACT scales half the blocks (Copy+scale-AP) while
  DVE ts_muls the rest, and the two output DMAs split over SP/ACT queues.

PSUM (8 banks): sc pool 2x[128,1024]f32 (4) + pv 1x (2) + aux 1x (2).
Section PV accumulators ping-pong pv/aux so boundaries have no PSUM
coupling; sec-0 setup projections and per-section finalize tiles reuse
whichever accumulator pool is idle at that point in the schedule.
"""
import numpy as np

B = 8
N = 4096
D = 128
NC = 32          # chunks of 128 along n/m
NSEC = 4         # sections of 1024 along n
SEC = 1024
DCOL = 640       # denominator cols on DVE (d_even); rest on GPSIMD (d_odd)
XSPL = 768       # exp split: ACT gets XSPL cols, DVE-Schraudolph the rest
                 # (window alternates sides per chunk to spread the error)
SCH_A = 184.6646884   # 2^7 * log2(e): Schraudolph exp in bf16 bit space
SCH_B = 16248.0       # (127 - c) * 2^7, calibrated on the real score range

_cache = {}


def _build(dcol=DCOL, warmup_mms=16, ptp_bufs=5):
    import concourse.bass as bass  # noqa
    import concourse.mybir as mybir
    import concourse.tile as tile
    from concourse import bacc

    F32 = mybir.dt.float32
    F32R = mybir.dt.float32r
    BF16 = mybir.dt.bfloat16
    I16 = mybir.dt.int16
    Exp = mybir.ActivationFunctionType.Exp
    Ident = mybir.ActivationFunctionType.Identity
    ADD = mybir.AluOpType.add
    MULT = mybir.AluOpType.mult
    ecol = SEC - dcol
    assert dcol % 128 == 0
    nde = dcol // 128

    nc = bacc.Bacc()
    xt = nc.declare_dram_parameter("xt", [D, N], F32, isOutput=False)
    wp = nc.declare_dram_parameter("wp", [128, 643], F32, isOutput=False)
    y = nc.declare_dram_parameter("y", [N, D], F32, isOutput=True)

    xt_dram = xt.rearrange("p (c l) -> p c l", l=128)
    y_dram = y.rearrange("(c p) d -> p c d", p=128)

    with tile.TileContext(nc) as tc:
        with (
            tc.tile_pool(name="big", bufs=1) as big,
            tc.tile_pool(name="pta", bufs=ptp_bufs) as ptpa,
            tc.tile_pool(name="ptb", bufs=ptp_bufs) as ptpb,
            tc.tile_pool(name="wrk", bufs=2) as wrk,
            tc.tile_pool(name="scA", bufs=2, space="PSUM") as ps_sca,
            tc.tile_pool(name="scB", bufs=2, space="PSUM") as ps_scb,
            tc.tile_pool(name="pv", bufs=1, space="PSUM") as ps_pv,
            tc.tile_pool(name="aux", bufs=1, space="PSUM") as ps_aux,
        ):
            # --- input DMAs, criticality-ordered -------------------------
            xg0a = big.tile([128, 4, 128], F32R, tag="xT0a")
            xg0b = big.tile([128, 4, 128], F32R, tag="xT0b")
            wf32 = big.tile([128, 512], F32, tag="wf32")
            wbig = big.tile([128, 512], F32R, tag="wbig")
            wrest = big.tile([128, 131], F32, tag="wrest")
            wsm = big.tile([128, 258], BF16, tag="wsm")
            # x tiles come in via gpsimd casting DMAs (swdge queue); the
            # weights via the SP queue as fp32 plus a DVE rounding copy
            nc.gpsimd.dma_start(xg0a[:], xt_dram[:, 0:4, :])
            nc.gpsimd.dma_start(xg0b[:], xt_dram[:, 4:8, :])
            # startup-critical wk/wq land in their own small DMA first
            nc.sync.dma_start(wf32[:, 0:256], wp[:, 0:256])
            nc.sync.dma_start(wrest[:], wp[:, 512:643])
            nc.sync.dma_start(wf32[:, 256:512], wp[:, 256:512])
            # rounding copies: critical wq/wk now; wv/ident after the
            # prologue's k-chain (they are only needed for v0 / finalize)
            nc.vector.tensor_copy(wbig[:, 128:256], wf32[:, 128:256])
            nc.vector.tensor_copy(wbig[:, 0:128], wf32[:, 0:128])
            wkT = wbig[:, 0:128]
            wqT = wbig[:, 128:256]
            wvT = wbig[:, 256:384]
            ident_bf = wsm[:, 0:128]
            ones_bf = wsm[:, 128:129]
            bv_bc = wrest[:, 1:129]
            bk = wrest[:, 129:130]
            bq = wrest[:, 130:131]
            ones_col = wrest[:, 0:1]

            xT_g = [None] * 4
            xT_g[0] = (xg0a, xg0b)

            def emit_dma_x(g):
                xg = big.tile([128, 8, 128], F32R, tag=f"xT{g}")
                nc.gpsimd.dma_start(xg[:], xt_dram[:, g * 8:(g + 1) * 8, :])
                xT_g[g] = xg

            emit_dma_x(1)

            wu = big.tile([128, 128], BF16, tag="warm")
            nc.vector.memset(wu[:], 1.0)
            wu_ps = ps_sca.tile([128, 512], F32, tag="scA")
            wu_psb = ps_scb.tile([128, 512], F32, tag="scB")
            for _ in range(warmup_mms):
                nc.tensor.matmul(wu_ps[:, 0:128], wu[:], wu[:],
                                 start=True, stop=True,
                                 skip_group_check=True)

            def ptile(pool):
                t = pool.tile([128, 1024], F32,
                              tag="pvt" if pool is ps_pv else "auxt")
                return t

            def xslab(g, half):
                xg = xT_g[g]
                if isinstance(xg, tuple):
                    return xg[half].rearrange("p c f -> p (c f)")
                return xg.rearrange("p c f -> p (c f)")[
                    :, half * 512:(half + 1) * 512]

            def xchunk(g, j):
                xg = xT_g[g]
                if isinstance(xg, tuple):
                    return xg[j // 4][:, j % 4, :]
                return xg[:, j, :]

            kT = [None] * 4
            qT = [None] * 4
            v_g = [None] * 4

            # --- projection helpers (setup); psum from an explicit pool --
            def emit_k(g, pool, split=False):
                if split:
                    kga = big.tile([128, 512], F32R, tag="kT0a")
                    kgb = big.tile([128, 512], F32R, tag="kT0b")
                    pst = ptile(pool)
                    nc.tensor.matmul(pst[:, 0:512], wkT, xslab(g, 0),
                                     start=True, stop=True)
                    nc.vector.tensor_scalar_add(kga[:], pst[:, 0:512], bk)
                    nc.tensor.matmul(pst[:, 512:1024], wkT, xslab(g, 1),
                                     start=True, stop=True)
                    nc.vector.tensor_scalar_add(kgb[:], pst[:, 512:1024], bk)
                    kT[g] = (kga, kgb)
                    return
                tg = big.tile([128, 1024], F32R, tag=f"kT{g}")
                pst = ptile(pool)
                nc.tensor.matmul(pst[:, 0:512], wkT, xslab(g, 0),
                                 start=True, stop=True)
                nc.tensor.matmul(pst[:, 512:1024], wkT, xslab(g, 1),
                                 start=True, stop=True)
                nc.scalar.activation(tg[:], pst[:], Ident, bias=bk)
                kT[g] = tg

            def v_bias_copy(vg, psv, lo, hi):
                # psv [m,j,d] + bv (broadcast along d) -> vg BF16
                n = hi - lo
                bvx = bv_bc[:, None, :].to_broadcast((128, n, 128))
                nc.vector.tensor_tensor(
                    vg[:, lo:hi, :], psv[:, lo:hi, :], bvx, ADD)

            def emit_v(g, pool):
                vg = big.tile([128, 8, 128], BF16, tag=f"v{g}")
                psv = ptile(pool)
                psv3 = psv.rearrange("p (c f) -> p c f", f=128)
                for j in range(8):
                    nc.tensor.matmul(
                        psv[:, j * 128:(j + 1) * 128], xchunk(g, j), wvT,
                        start=True, stop=True,
                    )
                v_bias_copy(vg, psv3, 0, 8)
                v_g[g] = vg

            def q_slice(mc):
                return qT[mc // 8][:, (mc % 8) * 128:(mc % 8 + 1) * 128]

            def v_chunk(mc):
                return v_g[mc // 8][:, mc % 8, :]

            # --- prologue ------------------------------------------------
            # q0[0:512] rushed into the warmup sc tile's spare columns so
            # nothing serializes behind the k projection's PSUM tile.
            qt0 = big.tile([128, 1024], F32R, tag="qT0")
            qT[0] = qt0
            nc.tensor.matmul(wu_psb[:, 0:128], wqT,
                             xslab(0, 0)[:, 0:128], start=True, stop=True)
            nc.vector.tensor_scalar_add(qt0[:, 0:128], wu_psb[:, 0:128], bq)
            nc.tensor.matmul(wu_psb[:, 128:512], wqT,
                             xslab(0, 0)[:, 128:512], start=True, stop=True)
            # k0 (ps_pv pool; released before sec-0 PV begins)
            kga = big.tile([128, 512], F32R, tag="kT0a")
            kgb = big.tile([128, 512], F32R, tag="kT0b")
            k0ps = ptile(ps_pv)
            nc.tensor.matmul(k0ps[:, 0:512], wkT, xslab(0, 0),
                             start=True, stop=True)
            nc.vector.tensor_scalar_add(kga[:], k0ps[:, 0:512], bk)
            kT[0] = (kga, kgb)
            # scores chunk 0 first half + exp
            q_sl0 = q_slice(0)
            s0a = ps_sca.tile([128, 512], F32, tag="scA")
            nc.tensor.matmul(s0a[:], q_sl0, kga[:], start=True, stop=True)
            pT0a_t = ptpa.tile([128, 512], I16, tag="pta")
            pT0b_t = ptpb.tile([128, 512], I16, tag="ptb")
            pT0 = (pT0a_t, pT0b_t)
            nc.scalar.activation(pT0a_t.bitcast(BF16)[:], s0a[:], Exp)
            # k0 second half + scores 0 second half (before v0: x0b path)
            nc.tensor.matmul(k0ps[:, 512:1024], wkT, xslab(0, 1),
                             start=True, stop=True)
            nc.vector.tensor_scalar_add(kgb[:], k0ps[:, 512:1024], bk)
            s0b = ps_scb.tile([128, 512], F32, tag="scB")
            nc.tensor.matmul(s0b[:], q_sl0, kgb[:], start=True, stop=True)
            nc.scalar.activation(pT0b_t.bitcast(BF16)[:], s0b[:], Exp)
            # deferred weight casts + v0 (needed by PV(0) at iteration 2);
            # q0 tail bias-adds follow (needed from scores chunk 1 onward)
            nc.vector.tensor_copy(wbig[:, 256:384], wf32[:, 256:384])
            nc.vector.tensor_copy(ident_bf, wf32[:, 384:512])
            nc.vector.memset(ones_bf, 1.0)
            emit_v(0, ps_aux)
            nc.vector.tensor_scalar_add(qt0[:, 128:512], wu_psb[:, 128:512], bq)
            # q0 tail (aux, after v0)
            q0ps = ptile(ps_aux)
            nc.tensor.matmul(q0ps[:, 512:1024], wqT, xslab(0, 1),
                             start=True, stop=True)
            nc.vector.tensor_scalar_add(qt0[:, 512:1024], q0ps[:, 512:1024], bq)

            # per-section state carried into the next section's stream
            fin_jobs = [None]   # list of (step_fn) queue for spread finalize

            # sec-0 setup schedule: (mc -> callable)
            setup_sched = {}

            def add_setup(mc, fn):
                setup_sched.setdefault(mc, []).append(fn)

            # sec-0 setup, two-phase: matmuls at an even chunk (PE slack),
            # the DVE copy/bias-add at the following odd chunks — odd
            # iterations carry a GPSIMD denominator, so DVE is free there.
            setup_state = {}

            def emit_q_mm(g):
                tg = big.tile([128, 1024], F32R, tag=f"qT{g}")
                pst = ptile(ps_aux)
                nc.tensor.matmul(pst[:, 0:512], wqT, xslab(g, 0),
                                 start=True, stop=True)
                nc.tensor.matmul(pst[:, 512:1024], wqT, xslab(g, 1),
                                 start=True, stop=True)
                setup_state[f"q{g}"] = (tg, pst)
                qT[g] = tg

            def emit_q_add(g):
                tg, pst = setup_state.pop(f"q{g}")
                nc.scalar.activation(tg[:], pst[:], Ident, bias=bq)

            def emit_k_mm(g):
                tg = big.tile([128, 1024], F32R, tag=f"kT{g}")
                pst = ptile(ps_aux)
                nc.tensor.matmul(pst[:, 0:512], wkT, xslab(g, 0),
                                 start=True, stop=True)
                nc.tensor.matmul(pst[:, 512:1024], wkT, xslab(g, 1),
                                 start=True, stop=True)
                setup_state[f"k{g}"] = (tg, pst)
                kT[g] = tg

            def emit_k_add(g):
                tg, pst = setup_state.pop(f"k{g}")
                nc.scalar.activation(tg[:], pst[:], Ident, bias=bk)

            def emit_v_mm(g, quarter):
                if quarter == 0:
                    vg = big.tile([128, 8, 128], BF16, tag=f"v{g}")
                    psv = ptile(ps_aux)
                    setup_state[f"v{g}"] = (vg, psv)
                    v_g[g] = vg
                else:
                    vg, psv = setup_state[f"v{g}"]
                for j in range(quarter * 2, quarter * 2 + 2):
                    nc.tensor.matmul(
                        psv[:, j * 128:(j + 1) * 128], xchunk(g, j), wvT,
                        start=True, stop=True,
                    )

            def emit_v_copy(g):
                vg, psv = setup_state.pop(f"v{g}")
                psv3 = psv.rearrange("p (c f) -> p c f", f=128)
                v_bias_copy(vg, psv3, 0, 8)

            add_setup(1, lambda: emit_q_mm(1))
            add_setup(1, lambda: emit_dma_x(2))
            add_setup(3, lambda: emit_q_add(1))
            add_setup(3, lambda: emit_dma_x(3))
            for s, qq in ((4, 0), (5, 1), (6, 2), (7, 3)):
                add_setup(s, lambda q=qq: emit_v_mm(1, q))
            add_setup(8, lambda: emit_v_copy(1))
            add_setup(9, lambda: emit_q_mm(2))
            add_setup(11, lambda: emit_q_add(2))
            for s, qq in ((12, 0), (13, 1), (14, 2), (15, 3)):
                add_setup(s, lambda q=qq: emit_v_mm(2, q))
            add_setup(16, lambda: emit_v_copy(2))
            add_setup(17, lambda: emit_q_mm(3))
            add_setup(19, lambda: emit_q_add(3))
            for s, qq in ((20, 0), (21, 1), (22, 2), (23, 3)):
                add_setup(s, lambda q=qq: emit_v_mm(3, q))
            add_setup(24, lambda: emit_v_copy(3))
            add_setup(25, lambda: emit_k_mm(1))
            add_setup(27, lambda: emit_k_add(1))

            # deferred k projections: k2 in sec 1, k3 in sec 2 (pool chosen
            # to be the one NOT accumulating that section's PV)
            def sec_pool(sec):
                return ps_pv if sec % 2 == 0 else ps_aux

            pT_prev = pT0
            last_pv_chain = [None]  # pending last-chunk PV+denoms of section

            for sec in range(NSEC):
                # column split everywhere: DVE cols [0:dcol] (bf16, two
                # ping-pong accumulators), GPS cols [dcol:] (fp32 acc).
                d_acc_a = wrk.tile([128, dcol], BF16, tag="dea")
                d_acc_b = wrk.tile([128, dcol], BF16, tag="deb")
                d_acc = [d_acc_a, d_acc_b]
                d_odd_a = wrk.tile([128, ecol], BF16, tag="doa")
                d_odd_b = wrk.tile([128, ecol], BF16, tag="dob")
                d_odd = [d_odd_a, d_odd_b]
                ps_pv_t = ptile(sec_pool(sec))

                def emit_scores(mc, sec=sec):
                    tA = ps_sca.tile([128, 512], F32, tag="scA")
                    tB = ps_scb.tile([128, 512], F32, tag="scB")
                    q_sl = q_slice(mc)
                    kg = kT[sec]
                    if isinstance(kg, tuple):
                        ka, kb = kg[0][:], kg[1][:]
                    else:
                        ka, kb = kg[:, 0:512], kg[:, 512:1024]
                    nc.tensor.matmul(tA[:], q_sl, ka, start=True, stop=True)
                    nc.tensor.matmul(tB[:], q_sl, kb, start=True, stop=True)
                    return (tA, tB)

                def dve_half(mc, sec=sec):
                    # which 512-col half (scores tile) DVE Schraudolphs;
                    # alternates sides so approximation error spreads evenly
                    if mc % 2 == 1:
                        return 'B' if (mc // 2) % 2 == 0 else 'A'
                    if sec > 0:
                        if mc == 8:
                            return 'B'
                        if mc == 24:
                            return 'A'
                    return None

                def emit_exp(ps_s, mc, split=False):
                    # each scores half lands in its own one-bank PSUM tile
                    # with exactly one reader (PSUM tiles serialize readers),
                    # and each pT half is a separate single-writer tile; on
                    # 18 of 32 chunks DVE Schraudolphs one half (bf16 bit
                    # pattern via fused mult+add, int16 convert-on-write)
                    # while ACT exps the other concurrently.
                    tA, tB = ps_s
                    pa = ptpa.tile([128, 512], I16, tag="pta")
                    pb = ptpb.tile([128, 512], I16, tag="ptb")
                    h = None if split else dve_half(mc)
                    if h == 'A':
                        nc.vector.tensor_scalar(
                            pa[:], tA[:], SCH_A, SCH_B, MULT, ADD)
                    else:
                        nc.scalar.activation(pa.bitcast(BF16)[:], tA[:], Exp)
                    if h == 'B':
                        nc.vector.tensor_scalar(
                            pb[:], tB[:], SCH_A, SCH_B, MULT, ADD)
                    else:
                        nc.scalar.activation(pb.bitcast(BF16)[:], tB[:], Exp)
                    return (pa, pb)

                def emit_pv(mc, pT, ps_pv_t=ps_pv_t):
                    pa, pb = pT
                    nc.tensor.matmul(
                        ps_pv_t[:, 0:512], v_chunk(mc), pa.bitcast(BF16)[:],
                        start=(mc == 0), stop=(mc == NC - 1),
                        skip_group_check=True,
                    )
                    nc.tensor.matmul(
                        ps_pv_t[:, 512:1024], v_chunk(mc), pb.bitcast(BF16)[:],
                        start=(mc == 0), stop=(mc == NC - 1),
                        skip_group_check=True,
                    )

                def emit_denom(mc, pT, d_odd=d_odd, d_acc=d_acc, sec=sec):
                    pa, pb = pT
                    pab = pa.bitcast(BF16)
                    pbb = pb.bitcast(BF16)
                    da = d_acc[mc % 2]
                    if mc < 2:
                        nc.vector.tensor_copy(da[:, 0:512], pab[:])
                        nc.vector.tensor_copy(
                            da[:, 512:dcol], pbb[:, 0:dcol - 512])
                    else:
                        nc.vector.tensor_tensor(
                            da[:, 0:512], da[:, 0:512], pab[:], ADD)
                        nc.vector.tensor_tensor(
                            da[:, 512:dcol], da[:, 512:dcol],
                            pbb[:, 0:dcol - 512], ADD)
                    do = d_odd[mc % 2]
                    if mc < 2:
                        nc.gpsimd.tensor_copy(do[:], pbb[:, dcol - 512:512])
                    else:
                        nc.gpsimd.tensor_tensor(
                            do[:], do[:], pbb[:, dcol - 512:512], ADD)

                # finalize of THIS section (runs spread over next section)
                def make_fin(sec=sec, ps_pv_t=ps_pv_t,
                             d_odd=d_odd, d_acc=d_acc):
                    st = {}
                    pool = sec_pool(sec)
                    last = (sec == NSEC - 1)

                    def get_oc():
                        if "oc" not in st:
                            oc = wrk.tile([128, 1024], BF16, tag="oc")
                            st["oc"] = oc
                        return st["oc"]

                    def f_ocopy_a():
                        oc = get_oc()
                        if last:
                            nc.scalar.copy(oc[:, 0:512], ps_pv_t[:, 0:512])
                        else:
                            nc.vector.tensor_copy(oc[:, 0:512], ps_pv_t[:, 0:512])

                    def f_ocopy_b():
                        oc = get_oc()
                        if last:
                            # ACT again: DVE is busy with the final denom adds
                            nc.scalar.copy(oc[:, 512:1024], ps_pv_t[:, 512:1024])
                        else:
                            nc.vector.tensor_copy(oc[:, 512:1024], ps_pv_t[:, 512:1024])

                    def emit_denom_mms(ft):
                        for nb in range(8):
                            sl = slice(nb * 128, (nb + 1) * 128)
                            if nb < nde:
                                nc.tensor.matmul(
                                    ft[:, nb:nb + 1], d_acc[0][:, sl], ones_bf,
                                    start=True, stop=False,
                                    skip_group_check=True,
                                )
                                nc.tensor.matmul(
                                    ft[:, nb:nb + 1], d_acc[1][:, sl], ones_bf,
                                    start=False, stop=True,
                                    skip_group_check=True,
                                )
                            else:
                                sl2 = slice((nb - nde) * 128, (nb - nde + 1) * 128)
                                nc.tensor.matmul(
                                    ft[:, nb:nb + 1], d_odd[0][:, sl2], ones_bf,
                                    start=True, stop=False,
                                    skip_group_check=True,
                                )
                                nc.tensor.matmul(
                                    ft[:, nb:nb + 1], d_odd[1][:, sl2], ones_bf,
                                    start=False, stop=True,
                                    skip_group_check=True,
                                )

                    def f_denom():
                        ft = ptile(pool)
                        st["ft"] = ft
                        emit_denom_mms(ft)
                        rc = wrk.tile([128, 8], F32, tag="rc")
                        nc.vector.reciprocal(rc[:], ft[:, 0:8])
                        st["rc"] = rc
                        og = big.tile([128, 8, 128], F32, tag=f"out{sec % 2}")
                        st["og"] = og

                    def tpo_view(ft, nb):
                        # bf16 [128,128] transpose target inside the f32 tile
                        lo = 64 + nb * 64
                        return ft[:, lo:lo + 64].bitcast(BF16)

                    def mk_block(nb):
                        def f_block():
                            ft = st["ft"]
                            oc = st["oc"]
                            rc = st["rc"]
                            og = st["og"]
                            sl = slice(nb * 128, (nb + 1) * 128)
                            tp = tpo_view(ft, nb)
                            nc.tensor.transpose(tp, oc[:, sl], ident_bf)
                            nc.vector.tensor_scalar_mul(
                                og[:, nb, :], tp, rc[:, nb:nb + 1])
                        return f_block

                    def mk_dma(quarter):
                        def f_dma():
                            og = st["og"]
                            lo, hi = quarter * 2, quarter * 2 + 2
                            nc.sync.dma_start(
                                y_dram[:, sec * 8 + lo:sec * 8 + hi, :],
                                og[:, lo:hi, :],
                            )
                        return f_dma

                    if not last:
                        steps = [f_ocopy_a, f_ocopy_b, f_denom]
                        for nb in range(8):
                            steps.append(mk_block(nb))
                            if nb % 2 == 1:
                                steps.append(mk_dma(nb // 2))
                        return steps

                    # last section: tight serial tail. Transposes land in the
                    # now-idle sc pool (bf16 views); the denom chain gets its
                    # own sc tile so it runs in parallel with the copies.
                    def mk_tr(half):
                        def f_tr():
                            pool = ps_sca if half == 0 else ps_scb
                            tpo = pool.tile([128, 512], F32,
                                            tag="scA" if half == 0 else "scB")
                            st[f"tpo{half}"] = tpo
                            oc = st["oc"]
                            for j in range(4):
                                nb = half * 4 + j
                                tp = tpo[:, j * 64:(j + 1) * 64].bitcast(BF16)
                                nc.tensor.transpose(
                                    tp, oc[:, nb * 128:(nb + 1) * 128],
                                    ident_bf)
                        return f_tr

                    def f_denom_last():
                        ftd = ps_sca.tile([128, 512], F32, tag="scA")
                        emit_denom_mms(ftd)
                        rc = wrk.tile([128, 8], F32, tag="rc")
                        nc.vector.reciprocal(rc[:], ftd[:, 0:8])
                        st["rc"] = rc
                        og = big.tile([128, 8, 128], F32, tag=f"out{sec % 2}")
                        st["og"] = og

                    def mk_scale_dve(nb):
                        def f_s():
                            tpo = st[f"tpo{nb // 4}"]
                            j = nb % 4
                            tp = tpo[:, j * 64:(j + 1) * 64].bitcast(BF16)
                            nc.vector.tensor_scalar_mul(
                                st["og"][:, nb, :], tp, st["rc"][:, nb:nb + 1])
                        return f_s

                    def mk_scale_act(nb):
                        def f_scale():
                            tpo = st[f"tpo{nb // 4}"]
                            j = nb % 4
                            tp = tpo[:, j * 64:(j + 1) * 64].bitcast(BF16)
                            nc.scalar.mul(
                                st["og"][:, nb, :], tp, st["rc"][:, nb:nb + 1])
                        return f_scale

                    def mk_dma4(half, eng):
                        def f_dma():
                            og = st["og"]
                            lo, hi = half * 4, half * 4 + 4
                            eng().dma_start(
                                y_dram[:, sec * 8 + lo:sec * 8 + hi, :],
                                og[:, lo:hi, :],
                            )
                        return f_dma

                    def mk_dma2(quarter, eng):
                        def f_dma():
                            og = st["og"]
                            lo, hi = quarter * 2, quarter * 2 + 2
                            eng().dma_start(
                                y_dram[:, sec * 8 + lo:sec * 8 + hi, :],
                                og[:, lo:hi, :],
                            )
                        return f_dma

                    # pipelined tail: first half's transpose/scale/DMA chase
                    # ocopy_a while ACT copies the second half
                    steps = [f_denom_last, f_ocopy_a, mk_tr(0), f_ocopy_b,
                             mk_scale_dve(0), mk_scale_dve(1),
                             mk_scale_act(2), mk_scale_act(3),
                             mk_dma2(0, lambda: nc.sync),
                             mk_tr(1),
                             mk_dma2(1, lambda: nc.scalar),
                             mk_scale_dve(4), mk_scale_dve(5),
                             mk_scale_act(6), mk_scale_act(7),
                             mk_dma2(2, lambda: nc.sync),
                             mk_dma2(3, lambda: nc.scalar)]
                    return steps

                # positions in the NEXT section where fin steps run
                fin_positions = [2, 3, 4, 6, 7, 8, 9, 10, 11, 12, 13, 14, 15,
                                 16, 17]

                # PV lags scores by 2 chunks so the PE always runs the next
                # scores matmul first — its completion sem then settles well
                # before ACT needs it, keeping exp cadence at engine rate.
                pT_hist = {}
                if sec == 0:
                    pT_hist[0] = pT_prev
                for mc in range(NC):
                    if not (mc == 0 and sec == 0):
                        ps_s = emit_scores(mc)
                    # carried-over PV/denom of the previous section
                    if mc <= 1 and last_pv_chain[0]:
                        last_pv_chain[0][mc]()
                    if mc >= 2:
                        emit_pv(mc - 2, pT_hist.pop(mc - 2))
                    if mc >= 1:
                        emit_denom(mc - 1, pT_hist[mc - 1])
                    if not (mc == 0 and sec == 0):
                        pT_hist[mc] = emit_exp(ps_s, mc)
                    if mc >= 1:
                        # spread: sec-0 setup / fin steps / deferred k
                        if sec == 0 and mc in setup_sched:
                            for fn in setup_sched[mc]:
                                fn()
                        if sec > 0 and fin_jobs[0]:
                            if mc in fin_positions:
                                idx = fin_positions.index(mc)
                                if idx < len(fin_jobs[0]):
                                    fin_jobs[0][idx]()
                        if sec == 1 and mc == 20:
                            emit_k(2, ps_pv)
                        elif sec == 2 and mc == 20:
                            emit_k(3, ps_aux)

                # pend the last two PVs + last denom into the next section
                def mk_last(p30=pT_hist[NC - 2], p31=pT_hist[NC - 1],
                            pv=emit_pv, dn=emit_denom):
                    def run0():
                        pv(NC - 2, p30)

                    def run1():
                        pv(NC - 1, p31)
                        dn(NC - 1, p31)
                    return [run0, run1]

                last_pv_chain[0] = mk_last()
                fin_jobs[0] = make_fin()

            # drain: last section's PV/denoms + its finalize immediately
            if last_pv_chain[0] is not None:
                for fn in last_pv_chain[0]:
                    fn()
                last_pv_chain[0] = None
            for step in fin_jobs[0]:
                step()

    nc.finalize()
    return nc


def _get_nc():
    if "nc" not in _cache:
        _cache["nc"] = _build()
    return _cache["nc"]


def make_wp(Wk, Wq, Wv, bk, bq, bv):
    # layout: startup-critical wk/wq first so their DMA can land alone
    wp = np.zeros((128, 643), np.float32)
    wp[:, 0:128] = Wk.T
    wp[:, 128:256] = Wq.T
    wp[:, 256:384] = Wv.T
    wp[:, 384:512] = np.eye(128, dtype=np.float32)
    wp[:, 512] = 1.0
    wp[:, 513:641] = np.broadcast_to(bv[None, :], (128, 128))
    wp[:, 641] = bk
    wp[:, 642] = bq
    return wp


def kernel(x, Wk, bk, Wq, bq, Wv, bv, **_ignored):
    from concourse.bass_utils import run_bass_kernel_spmd

    x = np.asarray(x, dtype=np.float32)
    wp = make_wp(
        np.asarray(Wk, np.float32), np.asarray(Wq, np.float32),
        np.asarray(Wv, np.float32), np.asarray(bk, np.float32),
        np.asarray(bq, np.float32), np.asarray(bv, np.float32),
    )

    nc = _get_nc()
    in_maps = [
        {"xt": np.ascontiguousarray(x[b].T), "wp": wp} for b in range(B)
    ]
    res = run_bass_kernel_spmd(nc, in_maps, core_ids=list(range(B)))
    out = np.stack([res.results[b]["y"] for b in range(B)], axis=0)
    return out
